# revision 1
# baseline (speedup 1.0000x reference)
"""AttentionPooling (segment softmax pooling) Trainium2 kernel.

Math (per reference):
    h = tanh(x @ W1 + b1); s = h @ W2 + b2
    w = softmax(s) within each contiguous segment (batch is sorted)
    out[b] = sum_{r in b} w_r * x[r]

Device algorithm (per core, segments sharded 512/core):
  Softmax is shift-invariant and |s| <= ||W2||_1 + |b2| ~ 9, so we skip the
  per-segment max and use e_r = exp(s_r + b2) directly (safe in fp32).
  out[b] = (sum e_r x_r) / (sum e_r): both sums come from one-hot matmuls
  contracted over rows, accumulated in PSUM over a 64-segment group window,
  then scatter-accumulated (indirect DMA, compute_op=add) into a DRAM
  scratch [segs, 257] (256 pooled cols + 1 sum col); a final pass divides.

  Scores need x^T (D on partitions): host supplies x^T in bf16 (score path
  only shapes softmax weights; bf16 there perturbs the output by ~1e-3
  relative). Pooling reads x in natural layout (dtype configurable).

The program is identical across cores (SPMD); all data-dependent segment
offsets travel through input tensors (batch_local window ids + scatter row
indices), never through baked constants.
"""

import os
from contextlib import ExitStack

import numpy as np
import ml_dtypes

LAST_EXEC_NS = None

import concourse.bass as bass
import concourse.bacc as bacc
import concourse.tile as tile
from concourse import mybir
from concourse.bass import IndirectOffsetOnAxis
from concourse.bass_utils import run_bass_kernel_spmd

# ---- problem constants (hardcoded per contract) ----
N_TOTAL = 500000
D = 256
H = 128
NUM_SEGMENTS = 4096
N_CORES = 8
SEGS_PER_CORE = NUM_SEGMENTS // N_CORES  # 512

G_ROWS = 2048          # rows per group
TILES_PER_G = 16       # 128-row tiles per group
SUB_PER_G = 4          # 512-row subtiles per group (score matmuls)
W_SEG = 64             # segment window width per group (host asserts fit)
SCRATCH_ROWS = 640     # 512 real segs + 128 pad rows for window overflow
PAD_BL = 255.0         # batch_local value for padding rows (never matches iota)

F32 = mybir.dt.float32
F32R = mybir.dt.float32r
BF16 = mybir.dt.bfloat16
I32 = mybir.dt.int32

# pooling input dtype: "f32" (safe) or "bf16" (halves pooling-read traffic)
X_POOL_DTYPE = "bf16"


def build_nc(n_groups: int, b2_val: float) -> bass.Bass:
    r_pad = n_groups * G_ROWS
    n_tiles = n_groups * TILES_PER_G
    xdt = F32 if X_POOL_DTYPE == "f32" else BF16

    nc = bacc.Bacc("TRN2", target_bir_lowering=False, debug=False)

    # DRAM I/O
    # esel/ones must match x's matmul dtype: f32r with f32 x, bf16 with bf16 x
    edt = F32R if xdt == F32 else BF16
    # x_nat carries D cols of x, a ones column (col 256, folds the seg_sum
    # matmul into the pooling matmul), and a zero pad col. Layout is
    # partition-major [128, n_tiles, 258]: x_nat[p, t, :] = x[128t + p, :],
    # so one group's load is a single contiguous 8.2KB run per partition.
    x_nat = nc.dram_tensor("x_nat", [128, n_tiles, D + 2], xdt, kind="ExternalInput")
    xT = nc.dram_tensor("xT", [D, r_pad], BF16, kind="ExternalInput")
    w1c = nc.dram_tensor("w1c", [2, 128, H], BF16, kind="ExternalInput")
    w2col = nc.dram_tensor("w2col", [H, 1], BF16, kind="ExternalInput")
    b1col = nc.dram_tensor("b1col", [H, 1], F32, kind="ExternalInput")
    iota64 = nc.dram_tensor("iota64", [128, W_SEG], BF16, kind="ExternalInput")
    bl_all = nc.dram_tensor("bl_all", [128, n_tiles], F32, kind="ExternalInput")
    seg_idx = nc.dram_tensor("seg_idx", [W_SEG, n_groups], I32, kind="ExternalInput")
    # ExternalOutput buffers are zero-initialized by the runtime — scratch
    # relies on that for its scatter-accumulate
    scratch = nc.dram_tensor("scratch", [SCRATCH_ROWS, 257], F32, kind="ExternalOutput")
    out = nc.dram_tensor("out", [SCRATCH_ROWS, D], F32, kind="ExternalOutput")

    with tile.TileContext(nc) as tc, ExitStack() as ctx:
        const_pool = ctx.enter_context(tc.tile_pool(name="const", bufs=1))
        xT_pool = ctx.enter_context(tc.tile_pool(name="xT", bufs=6))
        xnat_pool = ctx.enter_context(tc.tile_pool(name="xnat", bufs=8))
        h_pool = ctx.enter_context(tc.tile_pool(name="h", bufs=6))
        e_pool = ctx.enter_context(tc.tile_pool(name="e", bufs=4))
        esel_pool = ctx.enter_context(tc.tile_pool(name="esel", bufs=12))
        flush_pool = ctx.enter_context(tc.tile_pool(name="flush", bufs=2))
        fin_pool = ctx.enter_context(tc.tile_pool(name="fin", bufs=2))
        u_psum = ctx.enter_context(tc.tile_pool(name="u_ps", bufs=2, space="PSUM"))
        s_psum = ctx.enter_context(tc.tile_pool(name="s_ps", bufs=2, space="PSUM"))
        p_psum = ctx.enter_context(tc.tile_pool(name="p_ps", bufs=2, space="PSUM"))

        # ---- constants ----
        w1c_t = const_pool.tile([128, 2 * H], BF16, tag="w1c")
        nc.sync.dma_start(w1c_t[:, 0:H], w1c[0])
        nc.sync.dma_start(w1c_t[:, H : 2 * H], w1c[1])
        w2_t = const_pool.tile([H, 1], BF16, tag="w2")
        nc.sync.dma_start(w2_t[:], w2col[:, :])
        b1_t = const_pool.tile([H, 1], F32, tag="b1")
        nc.sync.dma_start(b1_t[:], b1col[:, :])
        iota_t = const_pool.tile([128, W_SEG], BF16, tag="iota")
        nc.sync.dma_start(iota_t[:], iota64[:, :])
        bl_t = const_pool.tile([128, n_tiles], F32, tag="bl")
        nc.sync.dma_start(bl_t[:], bl_all[:, :])
        sidx_t = const_pool.tile([W_SEG, n_groups], I32, tag="sidx")
        nc.sync.dma_start(sidx_t[:], seg_idx[:, :])

        # ---- main loop over row groups ----
        for g in range(n_groups):
            # scores: s_nat[p, c] = score(row 2048g + 128c + p); two PSUM
            # tiles (banks) so each half's exp can fire without waiting for
            # (or bank-serializing against) the other half's matmuls
            xt0 = xT_pool.tile([128, G_ROWS], BF16, tag="xt0")
            xt1 = xT_pool.tile([128, G_ROWS], BF16, tag="xt1")
            nc.sync.dma_start(xt0[:], xT[0:128, g * G_ROWS : (g + 1) * G_ROWS])
            nc.sync.dma_start(xt1[:], xT[128:256, g * G_ROWS : (g + 1) * G_ROWS])
            e_t = e_pool.tile([128, TILES_PER_G], F32, tag="e")
            for half in range(2):
                snat = s_psum.tile([128, 8], F32, tag=f"snat{half}")
                for ii in range(SUB_PER_G // 2):
                    i = 2 * half + ii
                    sl = slice(512 * i, 512 * (i + 1))
                    u = u_psum.tile([H, 512], F32, tag="u")
                    nc.tensor.matmul(u[:], w1c_t[:, 0:H], xt0[:, sl], start=True, stop=False)
                    nc.tensor.matmul(u[:], w1c_t[:, H : 2 * H], xt1[:, sl], start=False, stop=True)
                    h_t = h_pool.tile([H, 512], BF16, tag="h")
                    nc.scalar.activation(h_t[:], u[:], mybir.ActivationFunctionType.Tanh, bias=b1_t[:, 0:1])
                    for j in range(4):
                        lc = 4 * ii + j
                        nc.tensor.matmul(
                            snat[:, lc : lc + 1],
                            h_t[:, 128 * j : 128 * (j + 1)],
                            w2_t[:],
                            start=(lc == 0),
                            stop=(lc == 7),
                            skip_group_check=True,
                        )
                nc.scalar.activation(
                    e_t[:, 8 * half : 8 * (half + 1)],
                    snat[:],
                    mybir.ActivationFunctionType.Exp,
                    bias=float(b2_val),
                )

            # pooling: accumulate [64 segs, 256 pooled + 1 sum] over the group
            pooled = p_psum.tile([128, 257], F32, tag="pooled")
            xn = xnat_pool.tile([128, TILES_PER_G * (D + 2)], xdt, tag="xn")
            t0 = g * TILES_PER_G
            nc.scalar.dma_start(
                xn[:].rearrange("p (t d) -> p t d", d=D + 2),
                x_nat[:, t0 : t0 + TILES_PER_G, :],
            )
            for c in range(TILES_PER_G):
                t_abs = g * TILES_PER_G + c
                esel = esel_pool.tile([128, W_SEG], edt, tag="esel")
                eng = nc.vector if c % 2 == 0 else nc.gpsimd
                eng.tensor_scalar(
                    esel[:],
                    iota_t[:],
                    bl_t[:, t_abs : t_abs + 1],
                    e_t[:, c : c + 1],
                    mybir.AluOpType.is_equal,
                    mybir.AluOpType.mult,
                )
                rhs = xn[:, c * (D + 2) : c * (D + 2) + 257]
                if xdt == F32:
                    rhs = rhs.bitcast(F32R)
                nc.tensor.matmul(
                    pooled[0:W_SEG, 0:257], esel[:], rhs,
                    start=(c == 0), stop=(c == TILES_PER_G - 1),
                    skip_group_check=True,
                )
            # flush: psum -> sbuf -> scatter-accumulate into scratch rows
            fl = flush_pool.tile([W_SEG, 257], F32, tag="fl")
            nc.vector.tensor_copy(fl[:], pooled[0:W_SEG, :])
            nc.gpsimd.indirect_dma_start(
                scratch[:, :],
                IndirectOffsetOnAxis(ap=sidx_t[:, g : g + 1], axis=0),
                fl[:],
                None,
                compute_op=mybir.AluOpType.add,
            )

        # ---- final normalize: out[s] = scratch[s, :256] / scratch[s, 256] ----
        # gpsimd loads: Tile dep-tracks scratch (DRAM), and the resulting
        # fan-in waits on the scatter sems exceed the HWDGE wait slots
        for b in range(5):
            ft = fin_pool.tile([128, 257], F32, tag="ft")
            nc.gpsimd.dma_start(ft[:], scratch[128 * b : 128 * (b + 1), :])
            rec = fin_pool.tile([128, 1], F32, tag="rec")
            eps = fin_pool.tile([128, 1], F32, tag="eps")
            nc.vector.tensor_scalar(
                eps[:], ft[:, D : D + 1], 1e-30, None, mybir.AluOpType.add,
            )
            nc.vector.reciprocal(rec[:], eps[:])
            ot = fin_pool.tile([128, D], F32, tag="ot")
            nc.vector.tensor_scalar(
                ot[:], ft[:, 0:D], rec[:, 0:1], None, mybir.AluOpType.mult,
            )
            nc.sync.dma_start(out[128 * b : 128 * (b + 1), :], ot[:])

    return nc


def kernel(x, batch, W1, b1, W2, b2):
    x = np.asarray(x, dtype=np.float32)
    batch = np.asarray(batch)
    W1 = np.asarray(W1, dtype=np.float32)
    b1 = np.asarray(b1, dtype=np.float32)
    W2 = np.asarray(W2, dtype=np.float32)
    b2 = np.asarray(b2, dtype=np.float32)
    n, d = x.shape
    assert d == D

    bounds = np.searchsorted(batch, np.arange(NUM_SEGMENTS + 1))
    core_starts = [int(bounds[SEGS_PER_CORE * m]) for m in range(N_CORES + 1)]
    rows_per_core = [core_starts[m + 1] - core_starts[m] for m in range(N_CORES)]
    n_groups = max(1, int(np.ceil(max(rows_per_core) / G_ROWS)))
    r_pad = n_groups * G_ROWS
    n_tiles = n_groups * TILES_PER_G

    xdt = np.float32 if X_POOL_DTYPE == "f32" else ml_dtypes.bfloat16

    # shared constant inputs
    w1c = np.ascontiguousarray(
        W1.reshape(2, 128, H).astype(ml_dtypes.bfloat16)
    )
    w2col = np.ascontiguousarray(W2.reshape(H, 1).astype(ml_dtypes.bfloat16))
    b1col = np.ascontiguousarray(b1.reshape(H, 1))
    iota64 = np.broadcast_to(np.arange(W_SEG), (128, W_SEG)).astype(ml_dtypes.bfloat16)
    b2_val = float(b2.reshape(-1)[0])

    in_maps = []
    for m in range(N_CORES):
        rs, re = core_starts[m], core_starts[m + 1]
        rows = re - rs
        xm = x[rs:re]
        x_flat = np.zeros((r_pad, D + 2), dtype=xdt)
        x_flat[:rows, :D] = xm.astype(xdt)
        x_flat[:rows, D] = xdt(1.0)
        # partition-major: x_nat[p, t, :] = x_flat[128t + p, :]
        x_nat = np.ascontiguousarray(
            x_flat.reshape(n_tiles, 128, D + 2).transpose(1, 0, 2)
        )
        xT = np.zeros((D, r_pad), dtype=ml_dtypes.bfloat16)
        xT[:, :rows] = xm.T.astype(ml_dtypes.bfloat16)

        seg_local = (batch[rs:re] - SEGS_PER_CORE * m).astype(np.int64)
        assert seg_local.min() >= 0 and seg_local.max() < SEGS_PER_CORE

        bl = np.full((128, n_tiles), PAD_BL, dtype=np.float32)
        sidx = np.empty((W_SEG, n_groups), dtype=np.int32)
        for g in range(n_groups):
            lo = g * G_ROWS
            hi = min((g + 1) * G_ROWS, rows)
            if lo >= rows:
                s0 = SEGS_PER_CORE  # pad region
            else:
                s0 = 32 * (int(seg_local[lo]) // 32)
                span = int(seg_local[hi - 1]) - s0
                assert span < W_SEG, f"group seg span {span} >= {W_SEG}"
                rr = np.arange(lo, hi)
                p = rr % 128
                c = (rr % G_ROWS) // 128
                bl[p, g * TILES_PER_G + c] = (seg_local[lo:hi] - s0).astype(np.float32)
            sidx[:, g] = s0 + np.arange(W_SEG)
        in_maps.append(
            {
                "x_nat": x_nat,
                "xT": xT,
                "w1c": w1c,
                "w2col": w2col,
                "b1col": b1col,
                "iota64": iota64,
                "bl_all": bl,
                "seg_idx": sidx,
            }
        )

    nc = build_nc(n_groups, b2_val)
    if not nc.is_finalized():
        nc.finalize()
    trace = os.environ.get("KERNEL_TRACE", "0") == "1"
    kw = {}
    if trace:
        kw = dict(trace=True, tmpdir=os.environ.get("KERNEL_TRACE_DIR") or None)
    res = run_bass_kernel_spmd(nc, in_maps, core_ids=list(range(N_CORES)), **kw)
    global LAST_EXEC_NS
    LAST_EXEC_NS = res.exec_time_ns
    if trace:
        print(
            f"exec_time_ns={res.exec_time_ns} mean={res.mean_exec_time_ns} "
            f"max_core={res.max_exec_time_core_id}",
            flush=True,
        )
    outs = res.results

    full = np.empty((NUM_SEGMENTS, D), dtype=np.float32)
    for m in range(N_CORES):
        full[SEGS_PER_CORE * m : SEGS_PER_CORE * (m + 1)] = outs[m]["out"][
            :SEGS_PER_CORE
        ]
    return full



# revision 4
# speedup vs baseline: 1.9428x; 1.9428x over previous
"""AttentionPooling (segment softmax pooling) Trainium2 kernel.

Math (per reference):
    h = tanh(x @ W1 + b1); s = h @ W2 + b2
    w = softmax(s) within each contiguous segment (batch is sorted)
    out[b] = sum_{r in b} w_r * x[r]

Device algorithm (per core, segments sharded 512/core):
  Softmax is shift-invariant and |s| <= ||W2||_1 + |b2| ~ 9, so we skip the
  per-segment max and use e_r = exp(s_r + b2) directly (safe in fp32).
  out[b] = (sum e_r x_r) / (sum e_r): both sums come from one-hot matmuls
  contracted over rows, accumulated in PSUM over a W_SEG-segment group
  window, then scatter-accumulated (indirect DMA, compute_op=add) into a
  DRAM scratch [segs, 257] (256 pooled cols + 1 sum col); a final pass
  divides.

  Scores need x^T (D on partitions): host supplies x^T in bf16 (score path
  only shapes softmax weights; bf16 there perturbs the output by ~1e-3
  relative). Pooling reads x in natural layout (bf16).

  The esel one-hot selector for a whole 2048-row group is built in two
  group-wide elementwise ops using stride-0 broadcast access patterns
  (is_equal on gpsimd, multiply-by-e on DVE) rather than per-tile ops —
  per-instruction overhead on DVE/GpSimd dominated the previous version.

The program is identical across cores (SPMD); all data-dependent segment
offsets travel through input tensors (batch_local window ids + scatter row
indices), never through baked constants.
"""

import os
from contextlib import ExitStack

import numpy as np
import ml_dtypes

LAST_EXEC_NS = None

import concourse.bass as bass
import concourse.bacc as bacc
import concourse.tile as tile
from concourse import mybir
from concourse.bass import IndirectOffsetOnAxis, broadcast_tensor_aps
from concourse.bass_utils import run_bass_kernel_spmd

# ---- problem constants (hardcoded per contract) ----
N_TOTAL = 500000
D = 256
H = 128
NUM_SEGMENTS = 4096
N_CORES = 8
SEGS_PER_CORE = NUM_SEGMENTS // N_CORES  # 512

G_ROWS = 2048          # rows per group
TILES_PER_G = 16       # 128-row tiles per group
SUB_PER_G = 4          # 512-row subtiles per group (score matmuls)
SCRATCH_ROWS = 640     # 512 real segs + 128 pad rows for window overflow
PAD_BL = 255.0         # batch_local value for padding rows (never matches iota)

F32 = mybir.dt.float32
BF16 = mybir.dt.bfloat16
I32 = mybir.dt.int32


def build_nc(n_groups: int, b2_val: float, w_seg: int) -> bass.Bass:
    r_pad = n_groups * G_ROWS
    n_tiles = n_groups * TILES_PER_G

    nc = bacc.Bacc("TRN2", target_bir_lowering=False, debug=False)

    # DRAM I/O
    # x_nat carries D cols of x, a ones column (col 256, folds the seg_sum
    # matmul into the pooling matmul), and a zero pad col. Layout is
    # partition-major [128, n_tiles, 258]: x_nat[p, t, :] = x[128t + p, :],
    # so one group's load is a single contiguous 8.2KB run per partition.
    x_nat = nc.dram_tensor("x_nat", [128, n_tiles, D + 2], BF16, kind="ExternalInput")
    xT = nc.dram_tensor("xT", [D, r_pad], BF16, kind="ExternalInput")
    w1c = nc.dram_tensor("w1c", [2, 128, H], BF16, kind="ExternalInput")
    w2col = nc.dram_tensor("w2col", [H, 1], BF16, kind="ExternalInput")
    b1col = nc.dram_tensor("b1col", [H, 1], F32, kind="ExternalInput")
    iotaw = nc.dram_tensor("iotaw", [128, w_seg], BF16, kind="ExternalInput")
    bl_all = nc.dram_tensor("bl_all", [128, n_tiles], F32, kind="ExternalInput")
    seg_idx = nc.dram_tensor("seg_idx", [w_seg, n_groups], I32, kind="ExternalInput")
    # ExternalOutput buffers are zero-initialized by the runtime — scratch
    # relies on that for its scatter-accumulate
    scratch = nc.dram_tensor("scratch", [SCRATCH_ROWS, 257], F32, kind="ExternalOutput")
    out = nc.dram_tensor("out", [SCRATCH_ROWS, D], F32, kind="ExternalOutput")

    with tile.TileContext(nc) as tc, ExitStack() as ctx:
        const_pool = ctx.enter_context(tc.tile_pool(name="const", bufs=1))
        xT_pool = ctx.enter_context(tc.tile_pool(name="xT", bufs=6))
        xnat_pool = ctx.enter_context(tc.tile_pool(name="xnat", bufs=4))
        h_pool = ctx.enter_context(tc.tile_pool(name="h", bufs=2))
        e_pool = ctx.enter_context(tc.tile_pool(name="e", bufs=3))
        mask_pool = ctx.enter_context(tc.tile_pool(name="mask", bufs=3))
        esel_pool = ctx.enter_context(tc.tile_pool(name="esel", bufs=3))
        flush_pool = ctx.enter_context(tc.tile_pool(name="flush", bufs=2))
        fin_pool = ctx.enter_context(tc.tile_pool(name="fin", bufs=2))
        u_psum = ctx.enter_context(tc.tile_pool(name="u_ps", bufs=1, space="PSUM"))
        s_psum = ctx.enter_context(tc.tile_pool(name="s_ps", bufs=2, space="PSUM"))
        p_psum = ctx.enter_context(tc.tile_pool(name="p_ps", bufs=2, space="PSUM"))

        # ---- constants ----
        w1c_t = const_pool.tile([128, 2 * H], BF16, tag="w1c")
        nc.sync.dma_start(w1c_t[:, 0:H], w1c[0])
        nc.sync.dma_start(w1c_t[:, H : 2 * H], w1c[1])
        w2_t = const_pool.tile([H, 1], BF16, tag="w2")
        nc.sync.dma_start(w2_t[:], w2col[:, :])
        b1_t = const_pool.tile([H, 1], F32, tag="b1")
        nc.sync.dma_start(b1_t[:], b1col[:, :])
        iota_t = const_pool.tile([128, w_seg], BF16, tag="iota")
        nc.sync.dma_start(iota_t[:], iotaw[:, :])
        bl_t = const_pool.tile([128, n_tiles], F32, tag="bl")
        nc.sync.dma_start(bl_t[:], bl_all[:, :])
        sidx_t = const_pool.tile([w_seg, n_groups], I32, tag="sidx")
        nc.sync.dma_start(sidx_t[:], seg_idx[:, :])

        # per-group pipeline state carried across loop iterations
        prev = None  # (esel_tile, xn_tile)

        def flush_window(esel, xn, g):
            """Pooling matmuls for group g, then scatter-accumulate."""
            pooled = p_psum.tile([w_seg, 257], F32, tag="pooled")
            for c in range(TILES_PER_G):
                nc.tensor.matmul(
                    pooled[:, 0:257],
                    esel[:, c * w_seg : (c + 1) * w_seg],
                    xn[:, c * (D + 2) : c * (D + 2) + 257],
                    start=(c == 0),
                    stop=(c == TILES_PER_G - 1),
                    skip_group_check=True,
                )
            fl = flush_pool.tile([w_seg, 257], F32, tag="fl")
            nc.vector.tensor_copy(fl[:], pooled[:, :])
            nc.gpsimd.indirect_dma_start(
                scratch[:, :],
                IndirectOffsetOnAxis(ap=sidx_t[:, g : g + 1], axis=0),
                fl[:],
                None,
                compute_op=mybir.AluOpType.add,
            )

        # ---- main loop over row groups ----
        for g in range(n_groups):
            xt0 = xT_pool.tile([128, G_ROWS], BF16, tag="xt0")
            xt1 = xT_pool.tile([128, G_ROWS], BF16, tag="xt1")
            nc.sync.dma_start(xt0[:], xT[0:128, g * G_ROWS : (g + 1) * G_ROWS])
            nc.sync.dma_start(xt1[:], xT[128:256, g * G_ROWS : (g + 1) * G_ROWS])
            xn = xnat_pool.tile([128, TILES_PER_G * (D + 2)], BF16, tag="xn")
            t0 = g * TILES_PER_G
            nc.scalar.dma_start(
                xn[:].rearrange("p (t d) -> p t d", d=D + 2),
                x_nat[:, t0 : t0 + TILES_PER_G, :],
            )

            # scores: u_i = W1a^T xt0_i + W1b^T xt1_i per 512-row subtile,
            # ordered all-W1a then all-W1b so the stationary weight only
            # changes twice per group.
            u_tiles = [
                u_psum.tile([H, 512], F32, tag=f"u{i}", name=f"u{i}")
                for i in range(SUB_PER_G)
            ]
            for i in range(SUB_PER_G):
                sl = slice(512 * i, 512 * (i + 1))
                nc.tensor.matmul(
                    u_tiles[i][:], w1c_t[:, 0:H], xt0[:, sl],
                    start=True, stop=False, skip_group_check=True,
                )
            h_t = h_pool.tile([H, G_ROWS], BF16, tag="h")
            for i in range(SUB_PER_G):
                sl = slice(512 * i, 512 * (i + 1))
                nc.tensor.matmul(
                    u_tiles[i][:], w1c_t[:, H : 2 * H], xt1[:, sl],
                    start=False, stop=True, skip_group_check=True,
                )
                nc.scalar.activation(
                    h_t[:, sl], u_tiles[i][:],
                    mybir.ActivationFunctionType.Tanh, bias=b1_t[:, 0:1],
                )

            # interleave previous group's pooling here: it fills the PE
            # while tanh/exp/esel for this group run on Act/DVE/GpSimd.
            if prev is not None:
                flush_window(*prev)

            # snat[p, c] = score of row 128c + p (pre-bias)
            snat = s_psum.tile([128, TILES_PER_G], F32, tag="snat")
            for c in range(TILES_PER_G):
                nc.tensor.matmul(
                    snat[:, c : c + 1],
                    h_t[:, 128 * c : 128 * (c + 1)],
                    w2_t[:],
                    start=(c == 0),
                    stop=(c == TILES_PER_G - 1),
                    skip_group_check=True,
                )
            e_t = e_pool.tile([128, TILES_PER_G], F32, tag="e")
            nc.scalar.activation(
                e_t[:], snat[:], mybir.ActivationFunctionType.Exp, bias=float(b2_val)
            )

            # group-wide esel: mask[p,c,s] = (iota[s] == bl[p, t0+c]);
            # esel[p,c,s] = mask * e[p,c]. Broadcast via stride-0 APs.
            mask = mask_pool.tile([128, TILES_PER_G * w_seg], BF16, tag="mask")
            mask3 = mask[:].rearrange("p (c s) -> p c s", s=w_seg)
            iota3 = iota_t[:].rearrange("p (o s) -> p o s", o=1)
            bl3 = bl_t[:, t0 : t0 + TILES_PER_G].rearrange("p (c o) -> p c o", o=1)
            i_b, b_b = broadcast_tensor_aps(iota3, bl3)
            nc.vector.tensor_tensor(mask3, i_b, b_b, mybir.AluOpType.is_equal)
            esel = esel_pool.tile([128, TILES_PER_G * w_seg], BF16, tag="esel")
            esel3 = esel[:].rearrange("p (c s) -> p c s", s=w_seg)
            e3 = e_t[:].rearrange("p (c o) -> p c o", o=1)
            m_b, e_b = broadcast_tensor_aps(mask3, e3)
            nc.vector.tensor_tensor(esel3, m_b, e_b, mybir.AluOpType.mult)

            prev = (esel, xn, g)

        flush_window(*prev)

        # ---- final normalize: out[s] = scratch[s, :256] / scratch[s, 256] ----
        # gpsimd loads: Tile dep-tracks scratch (DRAM), and the resulting
        # fan-in waits on the scatter sems exceed the HWDGE wait slots
        for b in range(5):
            ft = fin_pool.tile([128, 257], F32, tag="ft")
            nc.gpsimd.dma_start(ft[:], scratch[128 * b : 128 * (b + 1), :])
            rec = fin_pool.tile([128, 1], F32, tag="rec")
            eps = fin_pool.tile([128, 1], F32, tag="eps")
            nc.vector.tensor_scalar(
                eps[:], ft[:, D : D + 1], 1e-30, None, mybir.AluOpType.add,
            )
            nc.vector.reciprocal(rec[:], eps[:])
            ot = fin_pool.tile([128, D], F32, tag="ot")
            nc.vector.tensor_scalar(
                ot[:], ft[:, 0:D], rec[:, 0:1], None, mybir.AluOpType.mult,
            )
            nc.sync.dma_start(out[128 * b : 128 * (b + 1), :], ot[:])

    return nc


def kernel(x, batch, W1, b1, W2, b2):
    x = np.asarray(x, dtype=np.float32)
    batch = np.asarray(batch)
    W1 = np.asarray(W1, dtype=np.float32)
    b1 = np.asarray(b1, dtype=np.float32)
    W2 = np.asarray(W2, dtype=np.float32)
    b2 = np.asarray(b2, dtype=np.float32)
    n, d = x.shape
    assert d == D

    bounds = np.searchsorted(batch, np.arange(NUM_SEGMENTS + 1))
    core_starts = [int(bounds[SEGS_PER_CORE * m]) for m in range(N_CORES + 1)]
    rows_per_core = [core_starts[m + 1] - core_starts[m] for m in range(N_CORES)]
    n_groups = max(1, int(np.ceil(max(rows_per_core) / G_ROWS)))
    r_pad = n_groups * G_ROWS
    n_tiles = n_groups * TILES_PER_G

    # window width: 32 segs if every group's span fits unaligned, else 64
    # (32-aligned start).
    max_span = 0
    for m in range(N_CORES):
        rs, re = core_starts[m], core_starts[m + 1]
        seg_local = batch[rs:re] - SEGS_PER_CORE * m
        rows = re - rs
        for g in range(n_groups):
            lo = g * G_ROWS
            hi = min((g + 1) * G_ROWS, rows)
            if lo < rows:
                max_span = max(
                    max_span, int(seg_local[hi - 1]) - int(seg_local[lo])
                )
    w_seg = 32 if max_span < 32 else 64

    # shared constant inputs
    w1c = np.ascontiguousarray(W1.reshape(2, 128, H).astype(ml_dtypes.bfloat16))
    w2col = np.ascontiguousarray(W2.reshape(H, 1).astype(ml_dtypes.bfloat16))
    b1col = np.ascontiguousarray(b1.reshape(H, 1))
    iotaw = np.broadcast_to(np.arange(w_seg), (128, w_seg)).astype(ml_dtypes.bfloat16)
    b2_val = float(b2.reshape(-1)[0])

    in_maps = []
    for m in range(N_CORES):
        rs, re = core_starts[m], core_starts[m + 1]
        rows = re - rs
        xm = x[rs:re]
        x_flat = np.zeros((r_pad, D + 2), dtype=ml_dtypes.bfloat16)
        x_flat[:rows, :D] = xm.astype(ml_dtypes.bfloat16)
        x_flat[:rows, D] = ml_dtypes.bfloat16(1.0)
        # partition-major: x_nat[p, t, :] = x_flat[128t + p, :]
        x_nat = np.ascontiguousarray(
            x_flat.reshape(n_tiles, 128, D + 2).transpose(1, 0, 2)
        )
        xT = np.zeros((D, r_pad), dtype=ml_dtypes.bfloat16)
        xT[:, :rows] = xm.T.astype(ml_dtypes.bfloat16)

        seg_local = (batch[rs:re] - SEGS_PER_CORE * m).astype(np.int64)
        assert seg_local.min() >= 0 and seg_local.max() < SEGS_PER_CORE

        bl = np.full((128, n_tiles), PAD_BL, dtype=np.float32)
        sidx = np.empty((w_seg, n_groups), dtype=np.int32)
        for g in range(n_groups):
            lo = g * G_ROWS
            hi = min((g + 1) * G_ROWS, rows)
            if lo >= rows:
                s0 = SEGS_PER_CORE  # pad region
            else:
                if w_seg == 32:
                    s0 = int(seg_local[lo])
                else:
                    s0 = 32 * (int(seg_local[lo]) // 32)
                span = int(seg_local[hi - 1]) - s0
                assert span < w_seg, f"group seg span {span} >= {w_seg}"
                rr = np.arange(lo, hi)
                p = rr % 128
                c = (rr % G_ROWS) // 128
                bl[p, g * TILES_PER_G + c] = (seg_local[lo:hi] - s0).astype(np.float32)
            sidx[:, g] = s0 + np.arange(w_seg)
        in_maps.append(
            {
                "x_nat": x_nat,
                "xT": xT,
                "w1c": w1c,
                "w2col": w2col,
                "b1col": b1col,
                "iotaw": iotaw,
                "bl_all": bl,
                "seg_idx": sidx,
            }
        )

    nc = build_nc(n_groups, b2_val, w_seg)
    if not nc.is_finalized():
        nc.finalize()
    trace = os.environ.get("KERNEL_TRACE", "0") == "1"
    kw = {}
    if trace:
        kw = dict(trace=True, tmpdir=os.environ.get("KERNEL_TRACE_DIR") or None)
    res = run_bass_kernel_spmd(nc, in_maps, core_ids=list(range(N_CORES)), **kw)
    global LAST_EXEC_NS
    LAST_EXEC_NS = res.exec_time_ns
    if trace:
        print(
            f"exec_time_ns={res.exec_time_ns} mean={res.mean_exec_time_ns} "
            f"max_core={res.max_exec_time_core_id}",
            flush=True,
        )
    outs = res.results

    full = np.empty((NUM_SEGMENTS, D), dtype=np.float32)
    for m in range(N_CORES):
        full[SEGS_PER_CORE * m : SEGS_PER_CORE * (m + 1)] = outs[m]["out"][
            :SEGS_PER_CORE
        ]
    return full
